# revision 42
# baseline (speedup 1.0000x reference)
"""Multi-head attention (B=8, S=1024, D=2048, H=16) on 8 Trainium2 NeuronCores.

Sharding: pure data parallel — core b computes batch element b. Weights are
replicated.

Layout strategy (v3):
  - q/k/v are transposed and cast to bf16 on the HOST, so the device never
    runs PE transposes for them. Weights are cast to bf16 on the host too
    (walrus rejects mixed 32/16-bit matmul operands; scores stay f32r).
  - Phase V: V = v @ Wv staged to DRAM in bf16, natural [s, d] layout.
  - Then per head: project KT_h/QT_h (SBUF-resident, no DRAM roundtrip)
    INTERLEAVED with the previous head's attention steps, so ACT exp
    latency hides under projection matmuls and the PE never starves.
  - Attention PV uses exp(scores) chunks as the STATIONARY operand and
    [V_h | ones] (bf16, 129 cols) as the moving operand: output arrives in
    natural [q, dh] layout with the softmax denominator as a free extra
    column — no sum matmuls, no output transposes. Each accumulation chain
    owns a full PSUM bank (start=True resets a bank's has_written flags, so
    two chains may not share one); q-chunks run in two passes per half.

Self-contained: builds the Bass program, shards inputs, runs SPMD via PJRT,
reassembles the full output.
"""
import numpy as np
from contextlib import ExitStack

import ml_dtypes

import concourse.bacc as bacc
import concourse.mybir as mybir
import concourse.tile as tile

B, S, D, H = 8, 1024, 2048, 16
DH = D // H            # 128
NK = D // 128          # 16 d-chunks
NS = S // 128          # 8 s-chunks
F32 = mybir.dt.float32
F32R = mybir.dt.float32r
BF16 = mybir.dt.bfloat16
SCALE = 1.0 / float(np.sqrt(DH))

_CACHE = {}


def round_f32r(x, drop=8):
    """Round-to-nearest-even dropping low `drop` mantissa bits (matches the
    precision the PE keeps for fp32r operands)."""
    b = np.ascontiguousarray(x, dtype=np.float32).view(np.uint32).astype(np.uint64)
    half = np.uint64(1 << (drop - 1))
    odd = (b >> np.uint64(drop)) & np.uint64(1)
    b = ((b + half - np.uint64(1) + odd) >> np.uint64(drop)) << np.uint64(drop)
    return b.astype(np.uint32).view(np.float32)


def build(opt=None, reps=1, timing=False, debug=None):
    _defaults = dict(
        ps512_bufs=4,        # PV psum tiles [128,512]
        sc_bufs=2,           # scores/projection ps1024 buffers
        e_bufs=11,
        wv_split=2,
        wq_split=2,
        vh_bufs=2,
        hh_bufs=4,           # SBUF qh/kh tiles in flight
        pipe_depth=2,        # score groups in flight ahead of PV
    )
    _defaults.update(opt or {})
    opt = _defaults
    nc = bacc.Bacc("TRN2", target_bir_lowering=False, debug=False)

    def _in(name, shape, dt_):
        if timing:
            return nc.dram_tensor(name, shape, dt_).ap()
        return nc.dram_tensor(name, shape, dt_, kind="ExternalInput").ap()

    # q/k/v arrive transposed: [D, S] bf16
    q_d = _in("q", [D, S], BF16)
    k_d = _in("k", [D, S], BF16)
    v_d = _in("v", [D, S], BF16)
    wq_d = _in("Wq", [D, D], BF16)
    wk_d = _in("Wk", [D, D], BF16)
    wv_d = _in("Wv", [D, D], BF16)
    if timing:
        out_d = nc.dram_tensor("out", [S, D], F32).ap()
        tout_d = nc.dram_tensor("tout", [1, 8], F32, kind="ExternalOutput").ap()
    else:
        out_d = nc.dram_tensor("out", [S, D], F32, kind="ExternalOutput").ap()

    vst = nc.dram_tensor("vst", [NS, 128, D], BF16)        # V[s,d] staged

    with tile.TileContext(nc) as tc, ExitStack() as ctx:
        psum = ctx.enter_context(tc.tile_pool(name="psum", bufs=opt["ps512_bufs"], space="PSUM"))
        const = ctx.enter_context(tc.tile_pool(name="const", bufs=1))

        onesf = const.tile([128, NS, 1], F32, name="onesf")
        nc.gpsimd.memset(onesf[:], 1.0)
        ones_bf = const.tile([128, NS, 1], BF16, name="ones_bf")
        nc.vector.tensor_copy(ones_bf[:], onesf[:])

        if timing:
            with tc.tile_pool(name="zfill", bufs=1) as zpool:
                zf = zpool.tile([128, S], F32, name="zfill")
                nc.gpsimd.memset(zf[:], 0.0)
                zb = zpool.tile([128, D], BF16, name="zfill_b")
                nc.vector.tensor_copy(zb[:, 0:S], zf[:])
                nc.vector.tensor_copy(zb[:, S:D], zf[:])
                for x in (q_d, k_d, v_d):
                    for i in range(NK):
                        nc.sync.dma_start(x[i * 128:(i + 1) * 128, :], zb[:, 0:S])
                for w in (wq_d, wk_d, wv_d):
                    for i in range(NK):
                        nc.sync.dma_start(w[i * 128:(i + 1) * 128, :], zb[:])

        for _rep in range(reps):
            _body_once(nc, tc, psum, const, ones_bf,
                       q_d, k_d, v_d, wq_d, wk_d, wv_d, out_d, vst, opt,
                       debug=debug)
        if timing:
            zo = const.tile([1, 8], F32, name="zo")
            nc.gpsimd.memset(zo[:], 0.0)
            nc.sync.dma_start(tout_d[:], zo[:])

    nc.compile()
    return nc


def _body_once(nc, tc, psum, const, ones_bf,
               q_d, k_d, v_d, wq_d, wk_d, wv_d, out_d, vst, opt, debug=None):
    with ExitStack() as outer:
        # xT pools live across phase V (prefetch) and the QK/attention loop
        pool_qT = outer.enter_context(tc.tile_pool(name="qT", bufs=1))
        pool_kT = outer.enter_context(tc.tile_pool(name="kT", bufs=1))
        pool_wqk = outer.enter_context(tc.tile_pool(name="wqk", bufs=4))

        # ---------------- Phase V: V = v @ Wv -> vst (bf16) ----------------
        with ExitStack() as pv:
            pool_vT = pv.enter_context(tc.tile_pool(name="vT", bufs=1))
            pool_w = pv.enter_context(tc.tile_pool(name="wv", bufs=2))
            pool_o = pv.enter_context(tc.tile_pool(name="vout", bufs=4))

            # wv slice 0 in quarter-chunks on the pool queue so the first
            # matmul chains start as soon as possible
            wv_tiles = {}
            src0 = wv_d[:, 0:512].rearrange("(kc p) n -> p kc n", p=128)
            wv0a = pool_w.tile([128, 4, 512], BF16, name="wv0a", bufs=1)
            nc.gpsimd.dma_start(wv0a[:, 0:1, :], src0[:, 0:1, :])
            nc.gpsimd.dma_start(wv0a[:, 1:4, :], src0[:, 1:4, :])
            # vT split across the sync and scalar queues
            vT = []
            for j in range(NK):
                t = pool_vT.tile([128, S], BF16, name=f"vT{j}")
                eng = nc.sync if j % 2 == 0 else nc.scalar
                eng.dma_start(t[:], v_d[j * 128:(j + 1) * 128, :])
                vT.append(t)
            wv0b = pool_w.tile([128, 6, 512], BF16, name="wv0b", bufs=1)
            wv0c = pool_w.tile([128, 6, 512], BF16, name="wv0c", bufs=1)
            for qtr in range(3):
                nc.gpsimd.dma_start(wv0b[:, qtr * 2:(qtr + 1) * 2, :],
                                    src0[:, 4 + qtr * 2:4 + (qtr + 1) * 2, :])
            for qtr in range(3):
                nc.gpsimd.dma_start(wv0c[:, qtr * 2:(qtr + 1) * 2, :],
                                    src0[:, 10 + qtr * 2:10 + (qtr + 1) * 2, :])
            # prefetch q/k transposed tiles on the pool-engine queue
            qT = [pool_qT.tile([128, S], BF16, name=f"qT{j}") for j in range(NK)]
            kT = [pool_kT.tile([128, S], BF16, name=f"kT{j}") for j in range(NK)]
            for j in range(NK):
                nc.gpsimd.dma_start(qT[j][:], q_d[j * 128:(j + 1) * 128, :])
                nc.gpsimd.dma_start(kT[j][:], k_d[j * 128:(j + 1) * 128, :])
            # preload the first Q/K weight slices on the scalar queue so the
            # QK phase starts without a DMA bubble
            w0q = pool_wqk.tile([128, NK, 256], BF16, name="wslice")
            w0k = pool_wqk.tile([128, NK, 256], BF16, name="wslice")
            for w_t, w_d in ((w0q, wq_d), (w0k, wk_d)):
                srcw = w_d[:, 0:256].rearrange("(kc p) n -> p kc n", p=128)
                nc.scalar.dma_start(w_t[:], srcw[:])

            for n in range(4):
                if n == 0:
                    wv_t = None
                elif n in wv_tiles:
                    wv_t = wv_tiles[n]
                else:
                    wv_t = pool_w.tile([128, NK, 512], BF16, name="wv")
                    src = wv_d[:, n * 512:(n + 1) * 512].rearrange("(kc p) n -> p kc n", p=128)
                    for qtr in range(opt["wv_split"]):
                        nq = NK // opt["wv_split"]
                        nc.sync.dma_start(wv_t[:, qtr * nq:(qtr + 1) * nq, :],
                                          src[:, qtr * nq:(qtr + 1) * nq, :])
                for m in range(NS):
                    ps = psum.tile([128, 512], F32, name="ps512")
                    for jk in range(NK):
                        if wv_t is not None:
                            mv = wv_t[:, jk, :]
                        elif jk < 4:
                            mv = wv0a[:, jk, :]
                        elif jk < 10:
                            mv = wv0b[:, jk - 4, :]
                        else:
                            mv = wv0c[:, jk - 10, :]
                        nc.tensor.matmul(ps[:], vT[jk][:, m * 128:(m + 1) * 128],
                                         mv,
                                         start=(jk == 0), stop=(jk == NK - 1))
                    vo = pool_o.tile([128, 512], BF16, name="vout")
                    nc.vector.tensor_copy(vo[:], ps[:])
                    (nc.sync if n < 2 else nc.gpsimd).dma_start(
                        vst.ap()[m, :, n * 512:(n + 1) * 512], vo[:])

        if debug == "v":
            with tc.tile_pool(name="dbg", bufs=4) as dbg:
                for m in range(NS):
                    tb = dbg.tile([128, D], BF16, name="dbgb")
                    nc.sync.dma_start(tb[:], vst.ap()[m])
                    tf = dbg.tile([128, D], F32, name="dbgf")
                    nc.vector.tensor_copy(tf[:], tb[:])
                    nc.sync.dma_start(out_d[m * 128:(m + 1) * 128, :], tf[:])
            return

        # ------------- Interleaved per-head QK projection + attention -------
        # PE stream per head h: attention steps of head h woven with the
        # projection matmuls of head h+1, so exp latency on ACT hides under
        # projection work and the PE never starves.
        with ExitStack() as ph:
            pool_w = pool_wqk
            pool_hh = ph.enter_context(tc.tile_pool(name="hh", bufs=opt["hh_bufs"]))
            pool_vh = ph.enter_context(tc.tile_pool(name="vh", bufs=opt["vh_bufs"]))
            pool_e = ph.enter_context(tc.tile_pool(name="e", bufs=opt["e_bufs"]))
            pool_os = ph.enter_context(tc.tile_pool(name="osb", bufs=3))
            pool_rs = ph.enter_context(tc.tile_pool(name="rs", bufs=3))

            def _load_w(ws, w_d):
                w_t = pool_w.tile([128, NK, 256], BF16, name="wslice")
                srcw = w_d[:, ws * 256:(ws + 1) * 256].rearrange("(kc p) n -> p kc n", p=128)
                for qtr in range(opt["wq_split"]):
                    nq = NK // opt["wq_split"]
                    nc.sync.dma_start(w_t[:, qtr * nq:(qtr + 1) * nq, :],
                                      srcw[:, qtr * nq:(qtr + 1) * nq, :])
                return w_t

            def _load_vh(h):
                vh1 = pool_vh.tile([128, NS, 132], BF16, name="vh")
                nc.scalar.dma_start(
                    vh1[:, :, 0:128],
                    vst.ap()[:, :, h * 128:(h + 1) * 128].rearrange("m p d -> p m d"))
                nc.vector.tensor_copy(vh1[:, :, 128:129], ones_bf[:])
                return vh1

            def _proj_units(wq_t, wk_t, hl):
                """qh/kh tiles plus 16 emission units (4 matmuls each) that
                project one head; chain order KT0, KT1, QT0, QT1."""
                kh = pool_hh.tile([128, S], F32R, name="hh")
                qh = pool_hh.tile([128, S], F32R, name="hh")
                units = []
                chains = ((wk_t, kT, kh, 0, "dve"), (wk_t, kT, kh, 1, "act"),
                          (wq_t, qT, qh, 0, "dve"), (wq_t, qT, qh, 1, "act"))
                for w_t, xT, xo, half, eng in chains:
                    box = {}

                    def unit(qtr, w_t=w_t, xT=xT, xo=xo, half=half, eng=eng, box=box):
                        if qtr == 0:
                            box["ps"] = psum.tile([128, 512], F32, name="pjps",
                                                  bufs=1)
                        ps = box["ps"]
                        for jk in range(qtr * 4, qtr * 4 + 4):
                            nc.tensor.matmul(
                                ps[:],
                                w_t[:, jk, hl * 128:(hl + 1) * 128],
                                xT[jk][:, half * 512:(half + 1) * 512],
                                start=(jk == 0), stop=(jk == NK - 1))
                        if qtr == 3:
                            cp = (nc.vector.tensor_copy if eng == "dve"
                                  else nc.scalar.copy)
                            cp(xo[:, half * 512:(half + 1) * 512], ps[:])

                    units.extend(lambda q=q, u=unit: u(q) for q in range(4))
                return qh, kh, units

            def _score_c(qh_, kh_, half, c):
                ps_sc = psum.tile([128, 512], F32, name="scps", bufs=3)
                nc.tensor.matmul(ps_sc[:],
                                 kh_[:, c * 128:(c + 1) * 128],
                                 qh_[:, half * 512:(half + 1) * 512],
                                 start=True, stop=True)
                e_t = pool_e.tile([128, 512], BF16, name="e")
                nc.scalar.activation(e_t[:], ps_sc[:],
                                     mybir.ActivationFunctionType.Exp,
                                     scale=SCALE)
                return e_t

            def _attn_ops(qh, kh, vh1, h):
                """Emission closures for one head: per half, pass 1 runs the
                scores/exp pipeline and accumulates q-chunks 0,1 (one PSUM
                bank per chain — a second start=True in a bank resets its
                has_written flags and loses the first chunk); pass 2 reuses
                the e tiles for q-chunks 2,3 and needs no ACT work."""
                ops = []
                for half in range(2):
                    st = {}

                    def p1(c, half=half, st=st):
                        if c == 0:
                            st["ps"] = [psum.tile([128, 512], F32, name="ps512")
                                        for _ in range(2)]
                            st["e"] = {0: _score_c(qh, kh, half, 0),
                                       1: _score_c(qh, kh, half, 1),
                                       2: _score_c(qh, kh, half, 2)}
                            st["rs"] = pool_rs.tile([128, 4], F32, name="rs")
                            st["o_t"] = pool_os.tile([128, 4, 128], F32,
                                                     name="osb")
                        if c + 3 < NS:
                            st["e"][c + 3] = _score_c(qh, kh, half, c + 3)
                        e_t = st["e"][c]
                        for qc in range(2):
                            nc.tensor.matmul(
                                st["ps"][qc][:, 0:129],
                                e_t[:, qc * 128:(qc + 1) * 128],
                                vh1[:, c, 0:129],
                                start=(c == 0), stop=(c == NS - 1))
                        if c == NS - 1:
                            _tail(st, 0)
                            nc.sync.dma_start(
                                out_d[half * 512:half * 512 + 256,
                                      h * 128:(h + 1) * 128]
                                .rearrange("(t p) d -> p t d", p=128),
                                st["o_t"][:, 0:2, :])

                    def p2(c, half=half, st=st):
                        if c == 0:
                            st["ps2"] = [psum.tile([128, 512], F32,
                                                   name="ps512")
                                         for _ in range(2)]
                        e_t = st["e"][c] if c < NS - 1 else st["e"].pop(c)
                        for qc in range(2):
                            nc.tensor.matmul(
                                st["ps2"][qc][:, 0:129],
                                e_t[:, (qc + 2) * 128:(qc + 3) * 128],
                                vh1[:, c, 0:129],
                                start=(c == 0), stop=(c == NS - 1))
                        if c == NS - 1:
                            _tail(st, 1)
                            nc.sync.dma_start(
                                out_d[half * 512 + 256:(half + 1) * 512,
                                      h * 128:(h + 1) * 128]
                                .rearrange("(t p) d -> p t d", p=128),
                                st["o_t"][:, 2:4, :])

                    def _tail(st, pair):
                        tiles = st["ps"] if pair == 0 else st["ps2"]
                        for i, ps in enumerate(tiles):
                            qc = pair * 2 + i
                            nc.vector.reciprocal(st["rs"][:, qc:qc + 1],
                                                 ps[:, 128:129])
                            nc.vector.tensor_scalar_mul(
                                st["o_t"][:, qc, :], ps[:, 0:128],
                                st["rs"][:, qc:qc + 1])

                    ops.append([lambda c=c, f=p1: f(c) for c in range(NS)])
                    ops.append([lambda c=c, f=p2: f(c) for c in range(NS)])
                if h == H - 1:
                    # no next-head projections to hide exp latency behind:
                    # use half0's ACT-free pass 2 as cover for half1's pass 1
                    p1h0, p2h0, p1h1, p2h1 = ops
                    flat = list(p1h0)
                    for a, b in zip(p2h0, p1h1):
                        flat.extend((a, b))
                    flat.extend(p2h1)
                    return flat
                return [f for grp in ops for f in grp]

            def _drive(attn, proj):
                attn = attn or []
                proj = proj or []
                if not attn:
                    for p in proj:
                        p()
                    return
                for i, a in enumerate(attn):
                    if i % 2 == 0 and i // 2 < len(proj):
                        proj[i // 2]()
                    a()
                for p in proj[(len(attn) + 1) // 2:]:
                    p()

            # prologue: head 0's projections run un-interleaved
            w_slices = {0: (w0q, w0k)}
            vh1_cur = _load_vh(0)
            qh, kh, units0 = _proj_units(w0q, w0k, 0)
            _drive(None, units0)
            cur = (qh, kh)
            for h in range(H):
                if h % 2 == 0 and h // 2 + 1 < 8:
                    w_slices[h // 2 + 1] = (_load_w(h // 2 + 1, wq_d),
                                            _load_w(h // 2 + 1, wk_d))
                nxt = None
                if h + 1 < H:
                    vh1_nxt = _load_vh(h + 1)
                    nwq, nwk = w_slices[(h + 1) // 2]
                    nqh, nkh, nunits = _proj_units(nwq, nwk, (h + 1) % 2)
                    nxt = (nqh, nkh)
                else:
                    vh1_nxt, nunits = None, None
                if debug == "vh" and h == 0:
                    with tc.tile_pool(name="dbgv", bufs=2) as dbg:
                        tf = dbg.tile([128, NS * 132], F32, name="dbgf")
                        nc.vector.tensor_copy(
                            tf[:].rearrange("p (m d) -> p m d", m=NS),
                            vh1_cur[:])
                        nc.sync.dma_start(out_d[0:128, 0:NS * 132], tf[:])
                    return
                if debug == "e" and h == 0:
                    # dump e chunks for head 0: rows c*128, cols half*512
                    with tc.tile_pool(name="dbge", bufs=4) as dbg:
                        for half in range(2):
                            for c in range(NS):
                                e_t = _score_c(cur[0], cur[1], half, c)
                                tf = dbg.tile([128, 512], F32, name="dbgf")
                                nc.vector.tensor_copy(tf[:], e_t[:])
                                nc.sync.dma_start(
                                    out_d[c * 128:(c + 1) * 128,
                                          half * 512:(half + 1) * 512], tf[:])
                    return
                if debug == "pv" and h == 0:
                    # accumulate PV for head 0, dump raw [q,129] regions
                    with tc.tile_pool(name="dbgp", bufs=4) as dbg:
                        for half in range(2):
                            ps_a = psum.tile([128, 512], F32, name="ps512")
                            ps_b = psum.tile([128, 512], F32, name="ps512")
                            ps_half = (ps_a, ps_a, ps_b, ps_b)
                            offs = (0, 256, 0, 256)
                            for c in range(NS):
                                e_t = _score_c(cur[0], cur[1], half, c)
                                for qc in range(4):
                                    nc.tensor.matmul(
                                        ps_half[qc][:, offs[qc]:offs[qc] + 129],
                                        e_t[:, qc * 128:(qc + 1) * 128],
                                        vh1_cur[:, c, 0:129],
                                        start=(c == 0), stop=(c == NS - 1))
                            for qc in range(4):
                                tf = dbg.tile([128, 129], F32, name="dbgf")
                                nc.vector.tensor_copy(
                                    tf[:], ps_half[qc][:, offs[qc]:offs[qc] + 129])
                                nc.sync.dma_start(
                                    out_d[half * 512 + qc * 128:
                                          half * 512 + (qc + 1) * 128, 0:129],
                                    tf[:])
                    return
                if debug in ("qh", "kh"):
                    _drive(None, nunits)
                    t = cur[0] if debug == "qh" else cur[1]
                    nc.sync.dma_start(
                        out_d[(h % 8) * 128:(h % 8 + 1) * 128,
                              (h // 8) * 1024:(h // 8 + 1) * 1024]
                        .bitcast(F32R),
                        t[:])
                else:
                    _drive(_attn_ops(cur[0], cur[1], vh1_cur, h), nunits)
                cur, vh1_cur = nxt, vh1_nxt


def _make_runner(nc, n_cores):
    """Jitted SPMD runner (q/k/v sharded over cores, weights replicated)."""
    import jax
    from jax.sharding import Mesh, PartitionSpec
    from jax.experimental.shard_map import shard_map
    from concourse import bass2jax
    from concourse.bass2jax import _bass_exec_p, install_neuronx_cc_hook

    install_neuronx_cc_hook()
    partition_name = nc.partition_id_tensor.name if nc.partition_id_tensor else None
    in_names, out_names, out_avals, zero_outs = [], [], [], []
    for alloc in nc.m.functions[0].allocations:
        if not isinstance(alloc, mybir.MemoryLocationSet):
            continue
        name = alloc.memorylocations[0].name
        if alloc.kind == "ExternalInput":
            if name != partition_name:
                in_names.append(name)
        elif alloc.kind == "ExternalOutput":
            out_names.append(name)
            shape = tuple(alloc.tensor_shape)
            dtype = mybir.dt.np(alloc.dtype)
            out_avals.append(jax.core.ShapedArray(shape, dtype))
            zero_outs.append(np.zeros(shape, dtype))
    sharded_in = {"q", "k", "v"}
    in_names_all = in_names + out_names
    if partition_name is not None:
        in_names_all.append(partition_name)

    def _body(*args):
        operands = list(args)
        if partition_name is not None:
            operands.append(bass2jax.partition_id_tensor())
        outs = _bass_exec_p.bind(
            *operands,
            out_avals=tuple(out_avals),
            in_names=tuple(in_names_all),
            out_names=tuple(out_names),
            lowering_input_output_aliases=(),
            sim_require_finite=True,
            sim_require_nnan=True,
            nc=nc,
        )
        return tuple(outs)

    devices = jax.devices()[:n_cores]
    mesh = Mesh(np.asarray(devices), ("core",))
    in_specs = tuple(
        PartitionSpec("core") if n in sharded_in else PartitionSpec()
        for n in in_names
    ) + (PartitionSpec("core"),) * len(out_names)
    out_specs = (PartitionSpec("core"),) * len(out_names)
    jitted = jax.jit(
        shard_map(_body, mesh=mesh, in_specs=in_specs, out_specs=out_specs,
                  check_rep=False),
        keep_unused=True,
    )

    def run(shared_map_, per_core_maps):
        import jax as _jax
        args = []
        for n in in_names:
            if n in sharded_in:
                args.append(np.concatenate([m[n] for m in per_core_maps], axis=0))
            else:
                args.append(shared_map_[n])
        concat_zeros = [
            np.zeros((n_cores * z.shape[0], *z.shape[1:]), z.dtype) for z in zero_outs
        ]
        out_arrs = jitted(*args, *concat_zeros)
        _jax.block_until_ready(out_arrs)
        return [
            {
                name: np.asarray(out_arrs[i]).reshape(n_cores, *out_avals[i].shape)[c]
                for i, name in enumerate(out_names)
            }
            for c in range(n_cores)
        ]

    return run


def _get_compiled():
    if "run" not in _CACHE:
        nc = build()
        _CACHE["run"] = _make_runner(nc, B)
    return _CACHE["run"]


def kernel(q, k, v, Wq, Wk, Wv):
    run = _get_compiled()
    bf = ml_dtypes.bfloat16
    shared = {
        "Wq": np.asarray(Wq, dtype=np.float32).astype(bf),
        "Wk": np.asarray(Wk, dtype=np.float32).astype(bf),
        "Wv": np.asarray(Wv, dtype=np.float32).astype(bf),
    }
    q = np.asarray(q, dtype=np.float32)
    k = np.asarray(k, dtype=np.float32)
    v = np.asarray(v, dtype=np.float32)
    per_core = [
        {
            "q": np.ascontiguousarray(q[b].astype(bf).T),
            "k": np.ascontiguousarray(k[b].astype(bf).T),
            "v": np.ascontiguousarray(v[b].astype(bf).T),
        }
        for b in range(B)
    ]
    results = run(shared, per_core)
    out = np.stack([results[b]["out"] for b in range(B)], axis=0)
    return out.astype(np.float32)


if __name__ == "__main__":
    rng = np.random.default_rng(0)
    qq = rng.standard_normal((B, S, D), dtype=np.float32)
    kk = rng.standard_normal((B, S, D), dtype=np.float32)
    vv = rng.standard_normal((B, S, D), dtype=np.float32)
    sc = np.float32(1.0 / np.sqrt(D))
    Wq = rng.standard_normal((D, D), dtype=np.float32) * sc
    Wk = rng.standard_normal((D, D), dtype=np.float32) * sc
    Wv = rng.standard_normal((D, D), dtype=np.float32) * sc
    o = kernel(q=qq, k=kk, v=vv, Wq=Wq, Wk=Wk, Wv=Wv)
    print("out", o.shape, o.dtype, np.abs(o).max())
